# revision 7
# baseline (speedup 1.0000x reference)
"""Causal attention block kernel for Trainium2 (Bass/Tile), SPMD over 8 cores.

Reference math (per batch b):
  q = x @ Wq; k = x @ Wk; v = x @ Wv      (biases are zero by spec)
  scores = (q @ k^T) * 1/sqrt(64), causal-masked
  att = softmax(scores)                    -> output 2 (B,H,T,T)
  y = att @ v ; out = y @ Wp               -> output 1 (B,T,C)

Sharding: data-parallel over batch B=128 -> 16 batches per core.

Layout strategy per core (all f32):
  xT (C,T) kept per batch (4 tiles [128,200]); built once with PE transposes,
  cached in DRAM across head-groups.
  qT/kT computed pair-of-heads at a time: psum [128(2*64), 200].
  v natural (T,C) per head: 2 psum tiles [tc,512].
  scores (t,s) in psum; softmax fused:
     tensor_tensor_reduce: out=(sc+mask)*(-scale), accum=min -> -rowmax
     ACT exp: exp(-1*in + (-rowmax...)) with accum_out -> rowsum
     reciprocal; ACT copy with scale=1/rowsum -> att
  att PE-transposed (3 blocks; the (t0,s1) block is exactly zero -> skipped)
  yT[c,t] = v^T-contraction matmuls; out[t,f] accumulates over heads in psum,
  group partials accumulated via DRAM staging.
"""

import numpy as np

import concourse.bass as bass
import concourse.mybir as mybir
import concourse.tile as tile
from concourse import bacc
from concourse.bass_utils import run_bass_kernel_spmd
from concourse.masks import make_identity

F32 = mybir.dt.float32

B, T, C = 128, 200, 512
H, K = 16, 64
N_CORES = 8
BPC = B // N_CORES          # 16 batches per core
NDCH = C // 128             # 4 contraction chunks of x features
SCALE = 1.0 / np.float32(np.sqrt(np.float32(K)))   # 0.125
# mask is applied in negated-score space: out = score*(-scale) + mask,
# so masked entries get +1e30 (huge positive -> exp(-(...)) == 0)
MASKVAL = 1.0e30
# t-chunks of the 200-long sequence on 128 partitions
TCH = [(0, 128), (128, 72)]
GSIZE = 4                   # heads per group (psum out accumulation span)
NG = H // GSIZE             # 4 groups


def build_nc(bpc: int = BPC) -> bass.Bass:
    nc = bacc.Bacc(None, target_bir_lowering=False)

    x = nc.dram_tensor("x", [bpc, T, C], F32, kind="ExternalInput")
    Wq = nc.dram_tensor("Wq", [C, H * K], F32, kind="ExternalInput")
    Wk = nc.dram_tensor("Wk", [C, H * K], F32, kind="ExternalInput")
    Wv = nc.dram_tensor("Wv", [C, H * C], F32, kind="ExternalInput")
    Wp = nc.dram_tensor("Wp", [H * C, C], F32, kind="ExternalInput")
    out = nc.dram_tensor("out", [bpc, T, C], F32, kind="ExternalOutput")
    att = nc.dram_tensor("att", [bpc, H, T, T], F32, kind="ExternalOutput")

    # DRAM views chunked for [128, ...] partition-major loads
    wq_v = Wq.rearrange("(n p) m -> p n m", p=128)   # [128, 4, 1024]
    wk_v = Wk.rearrange("(n p) m -> p n m", p=128)
    wv_v = Wv.rearrange("(n p) m -> p n m", p=128)   # [128, 4, 8192]
    wp_v = Wp.rearrange("(n p) m -> p n m", p=128)   # [128, 64, 512]

    with tile.TileContext(nc) as tc:
        with (
            tc.tile_pool(name="cst", bufs=1) as cst,
            tc.tile_pool(name="sb", bufs=2) as sb,
            tc.tile_pool(name="ps", bufs=1, space="PSUM") as ps,
            tc.tile_pool(name="dr", bufs=1, space="DRAM") as dr,
        ):
            # ---- constants ----
            ident = cst.tile([128, 128], F32)
            make_identity(nc, ident)
            # mask0: rows t=0..127 ; mask1: rows t=128..199 (only 72 used)
            mask0 = cst.tile([128, T], F32)
            nc.gpsimd.memset(mask0, 0.0)
            nc.gpsimd.affine_select(
                out=mask0, in_=mask0, compare_op=mybir.AluOpType.is_ge,
                fill=MASKVAL, base=0, pattern=[[-1, T]], channel_multiplier=1,
            )
            mask1 = cst.tile([128, T], F32)
            nc.gpsimd.memset(mask1, 0.0)
            nc.gpsimd.affine_select(
                out=mask1, in_=mask1, compare_op=mybir.AluOpType.is_ge,
                fill=MASKVAL, base=128, pattern=[[-1, T]], channel_multiplier=1,
            )
            masks = [mask0, mask1]

            # Warm-up ops: make PE and DVE observe the gpsimd (Pool) semaphore
            # once, so later transposes/ops need at most one extra sync wait
            # (walrus rejects transpose-mode matmuls with >1 wait command).
            warm = ps.tile([128, 128], F32, tag="attT", bufs=1, name="warmtp")
            nc.tensor.transpose(warm, ident, ident)
            dvetouch = cst.tile([128, 1], F32, name="dvetouch")
            nc.vector.tensor_tensor(
                out=dvetouch, in0=mask0[:, 0:1], in1=mask1[:, 0:1],
                op=mybir.AluOpType.add)

            # DRAM staging tiles (tracked by Tile for RAW/WAR deps)
            xtc = [dr.tile([NDCH, 128, T], F32, tag=f"xtc{b}", name=f"xtc{b}")
                   for b in range(bpc)]
            acc0 = [dr.tile([128, C], F32, tag=f"acc0_{b}", name=f"acc0_{b}")
                    for b in range(bpc)]
            acc1 = [dr.tile([72, C], F32, tag=f"acc1_{b}", name=f"acc1_{b}")
                    for b in range(bpc)]

            for g in range(NG):
                # ---- group weights: q/k for 4 heads (2 pairs) ----
                wq_g = sb.tile([128, NDCH, GSIZE * K], F32, tag="wq", bufs=2,
                               name=f"wq_g{g}")
                nc.sync.dma_start(
                    out=wq_g, in_=wq_v[:, :, g * GSIZE * K:(g + 1) * GSIZE * K])
                wk_g = sb.tile([128, NDCH, GSIZE * K], F32, tag="wk", bufs=2,
                               name=f"wk_g{g}")
                nc.sync.dma_start(
                    out=wk_g, in_=wk_v[:, :, g * GSIZE * K:(g + 1) * GSIZE * K])
                # per-head v/p weights, rolling buffers
                wv_h, wp_h = [], []
                for hi in range(GSIZE):
                    h = g * GSIZE + hi
                    wvt = sb.tile([128, NDCH, C], F32, tag="wv", bufs=5,
                                  name=f"wv_h{h}")
                    nc.sync.dma_start(out=wvt, in_=wv_v[:, :, h * C:(h + 1) * C])
                    wv_h.append(wvt)
                    wpt = sb.tile([128, NDCH, C], F32, tag="wp", bufs=5,
                                  name=f"wp_h{h}")
                    nc.sync.dma_start(out=wpt, in_=wp_v[:, h * NDCH:(h + 1) * NDCH, :])
                    wp_h.append(wpt)

                for b in range(bpc):
                    # ---- xT: [128, 4, 200] feature-major activations ----
                    xT = sb.tile([128, NDCH, T], F32, tag="xT", bufs=2,
                                 name=f"xT_g{g}b{b}")
                    if g == 0:
                        xn = sb.tile([128, 2, C], F32, tag="xn", bufs=2,
                                     name=f"xn_b{b}")
                        nc.sync.dma_start(out=xn[:, 0, :], in_=x[b, 0:128, :])
                        nc.sync.dma_start(out=xn[0:72, 1, :], in_=x[b, 128:200, :])
                        for d in range(NDCH):
                            xtp = ps.tile([128, T], F32, tag="yt", bufs=1,
                                          name=f"xtp_b{b}d{d}")
                            dsl = slice(d * 128, (d + 1) * 128)
                            nc.tensor.transpose(
                                xtp[:, 0:128], xn[:, 0, dsl], ident)
                            nc.tensor.transpose(
                                xtp[:, 128:200], xn[0:72, 1, dsl], ident[0:72, 0:72])
                            nc.scalar.copy(xT[:, d, :], xtp)
                        nc.sync.dma_start(
                            out=xtc[b].rearrange("d p t -> p d t"), in_=xT)
                    else:
                        nc.sync.dma_start(
                            out=xT, in_=xtc[b].rearrange("d p t -> p d t"))

                    # ---- psum accumulators for out[t, f] over this group ----
                    pout0 = ps.tile([128, C], F32, tag="out0", bufs=1,
                                    name=f"pout0_g{g}b{b}")
                    pout1 = ps.tile([72, C], F32, tag="out1", bufs=1,
                                    name=f"pout1_g{g}b{b}")
                    pouts = [pout0, pout1]

                    qk_sb = None
                    for hi in range(GSIZE):
                        h = g * GSIZE + hi
                        pair, sub = divmod(hi, 2)
                        if sub == 0:
                            # ---- qT/kT for the head pair: [128, 200] each ----
                            qkT = ps.tile([128, 2, T], F32, tag="qk", bufs=1,
                                          name=f"qkT_g{g}b{b}p{pair}")
                            psl = slice(pair * 128, (pair + 1) * 128)
                            for d in range(NDCH):
                                nc.tensor.matmul(
                                    qkT[:, 0, :], wq_g[:, d, psl], xT[:, d, :],
                                    start=(d == 0), stop=(d == NDCH - 1))
                            for d in range(NDCH):
                                nc.tensor.matmul(
                                    qkT[:, 1, :], wk_g[:, d, psl], xT[:, d, :],
                                    start=(d == 0), stop=(d == NDCH - 1))
                            qk_sb = sb.tile([128, 2, T], F32, tag="qksb", bufs=2,
                                            name=f"qksb_g{g}b{b}p{pair}")
                            nc.scalar.copy(qk_sb, qkT)

                        hp = slice(sub * K, (sub + 1) * K)

                        # ---- v natural [t, c] per t-chunk ----
                        v_sb = []
                        for ci, (t0, tsz) in enumerate(TCH):
                            vps = ps.tile([tsz, C], F32, tag="v", bufs=2,
                                          name=f"vps_g{g}b{b}h{hi}c{ci}")
                            tsl = slice(t0, t0 + tsz)
                            for d in range(NDCH):
                                nc.tensor.matmul(
                                    vps, xT[:, d, tsl], wv_h[hi][:, d, :],
                                    start=(d == 0), stop=(d == NDCH - 1))
                            vt = sb.tile([tsz, C], F32, tag=f"v{ci}", bufs=2,
                                         name=f"vsb_g{g}b{b}h{hi}c{ci}")
                            nc.scalar.copy(vt, vps)
                            v_sb.append(vt)

                        # ---- scores ----
                        sc = ps.tile([128, 2, T], F32, tag="sc", bufs=1,
                                     name=f"sc_g{g}b{b}h{hi}")
                        for ci, (t0, tsz) in enumerate(TCH):
                            nc.tensor.matmul(
                                sc[0:tsz, ci, :],
                                qk_sb[hp, 0, t0:t0 + tsz],
                                qk_sb[hp, 1, :],
                                start=True, stop=True)

                        # ---- softmax (rows on partitions, reduce on free) ----
                        scm = sb.tile([128, 2, T], F32, tag="scm", bufs=2,
                                      name=f"scm_g{g}b{b}h{hi}")
                        ate = sb.tile([128, 2, T], F32, tag="ate", bufs=2,
                                      name=f"ate_g{g}b{b}h{hi}")
                        atn = sb.tile([128, 2, T], F32, tag="atn", bufs=2,
                                      name=f"atn_g{g}b{b}h{hi}")
                        for ci, (t0, tsz) in enumerate(TCH):
                            nmax = sb.tile([128, 1], F32, tag="nmax", bufs=4,
                                           name=f"nmax_g{g}b{b}h{hi}c{ci}")
                            rsum = sb.tile([128, 1], F32, tag="rsum", bufs=4,
                                           name=f"rsum_g{g}b{b}h{hi}c{ci}")
                            rrec = sb.tile([128, 1], F32, tag="rrec", bufs=4,
                                           name=f"rrec_g{g}b{b}h{hi}c{ci}")
                            # scm = sc*(-scale) + mask ; nmax = min(scm) = -scale*rowmax
                            nc.vector.scalar_tensor_tensor(
                                scm[0:tsz, ci, :],
                                sc[0:tsz, ci, :],
                                float(-SCALE),
                                masks[ci][0:tsz, :],
                                mybir.AluOpType.mult,
                                mybir.AluOpType.add,
                            )
                            nc.vector.tensor_reduce(
                                nmax[0:tsz], scm[0:tsz, ci, :],
                                axis=mybir.AxisListType.X,
                                op=mybir.AluOpType.min,
                            )
                            # exp(scale*(sc) - rowmax_scaled); rowsum accumulated
                            nc.scalar.activation(
                                ate[0:tsz, ci, :], scm[0:tsz, ci, :],
                                mybir.ActivationFunctionType.Exp,
                                bias=nmax[0:tsz], scale=-1.0,
                                accum_out=rsum[0:tsz])
                            nc.vector.reciprocal(rrec[0:tsz], rsum[0:tsz])
                            nc.scalar.activation(
                                atn[0:tsz, ci, :], ate[0:tsz, ci, :],
                                mybir.ActivationFunctionType.Copy,
                                bias=0.0, scale=rrec[0:tsz])
                            nc.sync.dma_start(
                                out=att[b, h, t0:t0 + tsz, :],
                                in_=atn[0:tsz, ci, :])

                        # ---- att^T via PE transposes (skip zero block t0,s1) ----
                        tp = ps.tile([128, 272], F32, tag="attT", bufs=1,
                                     name=f"tp_g{g}b{b}h{hi}")
                        nc.tensor.transpose(
                            tp[:, 0:128], atn[:, 0, 0:128], ident)
                        nc.tensor.transpose(
                            tp[:, 128:200], atn[0:72, 1, 0:128], ident[0:72, 0:72])
                        nc.tensor.transpose(
                            tp[0:72, 200:272], atn[0:72, 1, 128:200],
                            ident[0:72, 0:72])
                        att_t = sb.tile([128, 272], F32, tag="attTs", bufs=2,
                                        name=f"attTs_g{g}b{b}h{hi}")
                        nc.vector.tensor_copy(att_t[:, 0:200], tp[:, 0:200])
                        nc.scalar.copy(att_t[0:72, 200:272], tp[0:72, 200:272])

                        # ---- yT[c, t] = sum_s v[s,c] attT[s,t] ----
                        yt_sb = sb.tile([128, 2, 2, T], F32, tag="yts", bufs=2,
                                        name=f"yts_g{g}b{b}h{hi}")
                        for cpair in range(2):
                            ytp = ps.tile([128, 2, T], F32, tag="yt", bufs=1,
                                          name=f"ytp_g{g}b{b}h{hi}p{cpair}")
                            for cc in range(2):
                                c = cpair * 2 + cc
                                csl = slice(c * 128, (c + 1) * 128)
                                nc.tensor.matmul(
                                    ytp[:, cc, :], v_sb[0][:, csl],
                                    att_t[:, 0:200],
                                    start=True, stop=False)
                                nc.tensor.matmul(
                                    ytp[:, cc, 128:200], v_sb[1][0:72, csl],
                                    att_t[0:72, 200:272],
                                    start=False, stop=True)
                            nc.vector.tensor_copy(yt_sb[:, cpair, :, :], ytp)

                        # ---- out[t, f] += yT^T @ Wp_h  (psum across group) ----
                        for ci, (t0, tsz) in enumerate(TCH):
                            tsl = slice(t0, t0 + tsz)
                            for c in range(4):
                                nc.tensor.matmul(
                                    pouts[ci],
                                    yt_sb[:, c // 2, c % 2, tsl],
                                    wp_h[hi][:, c, :],
                                    start=(hi == 0 and c == 0),
                                    stop=(hi == GSIZE - 1 and c == 3))

                    # ---- fold group partial into accumulator ----
                    accw = sb.tile([128, 2, C], F32, tag="accw", bufs=2,
                                   name=f"accw_g{g}b{b}")
                    if g == 0:
                        nc.scalar.copy(accw[:, 0, :], pout0)
                        nc.scalar.copy(accw[0:72, 1, :], pout1)
                    else:
                        nc.sync.dma_start(out=accw[:, 0, :], in_=acc0[b])
                        nc.sync.dma_start(out=accw[0:72, 1, :], in_=acc1[b])
                        nc.vector.tensor_tensor(
                            out=accw[:, 0, :], in0=pout0, in1=accw[:, 0, :],
                            op=mybir.AluOpType.add)
                        nc.vector.tensor_tensor(
                            out=accw[0:72, 1, :], in0=pout1, in1=accw[0:72, 1, :],
                            op=mybir.AluOpType.add)
                    if g == NG - 1:
                        nc.sync.dma_start(out=out[b, 0:128, :], in_=accw[:, 0, :])
                        nc.sync.dma_start(out=out[b, 128:200, :],
                                          in_=accw[0:72, 1, :])
                    else:
                        nc.sync.dma_start(out=acc0[b], in_=accw[:, 0, :])
                        nc.sync.dma_start(out=acc1[b], in_=accw[0:72, 1, :])

    nc.compile()
    return nc


_NC_CACHE: dict = {}


def _get_nc(bpc: int) -> bass.Bass:
    if bpc not in _NC_CACHE:
        _NC_CACHE[bpc] = build_nc(bpc)
    return _NC_CACHE[bpc]


def kernel(**inputs) -> tuple:
    x = np.ascontiguousarray(np.asarray(inputs["x"], dtype=np.float32))
    Wq = np.ascontiguousarray(np.asarray(inputs["Wq"], dtype=np.float32))
    Wk = np.ascontiguousarray(np.asarray(inputs["Wk"], dtype=np.float32))
    Wv = np.ascontiguousarray(np.asarray(inputs["Wv"], dtype=np.float32))
    Wp = np.ascontiguousarray(np.asarray(inputs["Wp"], dtype=np.float32))

    nc = _get_nc(BPC)
    in_maps = []
    for c in range(N_CORES):
        in_maps.append({
            "x": x[c * BPC:(c + 1) * BPC],
            "Wq": Wq, "Wk": Wk, "Wv": Wv, "Wp": Wp,
        })
    res = run_bass_kernel_spmd(nc, in_maps, core_ids=list(range(N_CORES)))
    out = np.concatenate([r["out"] for r in res.results], axis=0)
    att = np.concatenate([r["att"] for r in res.results], axis=0)
    return (out, att)
